# revision 1
# baseline (speedup 1.0000x reference)
"""Trainium2 Bass kernel for CombinedRepeatCausalLinear (parallel forward).

Computes out[b,e,t] = sum_s x[b,e,s] * W[s,t] + bias[t] where
  W[s,t] = mask(t>=s) * (w0[s]*d0^(t-s) + w1[t]*d1^(t-s))
for S = 2048, x of shape (8, 1024, 2048) fp32.

Strategy (8 NeuronCores, data-parallel over batch):
  - core c gets x[c] (1024 rows); host pre-transposes to xT (2048, 1024) so
    the contraction dim lands on SBUF partitions with contiguous DMAs.
  - W is rank-2 before causal masking: each (128 s) x (512 t) chunk of W is
    generated ON-CHIP by a K=2 matmul from tiny host-precomputed factor
    vectors (per-chunk exponent offsets keep fp32 in range), then boundary
    chunks are multiplied by one of 4 precomputed 0/1 causal masks on DVE.
  - main matmul runs in float32r (full-rate fp32 mode, 1 cyc/row at N=512):
    outT[t,r] = sum_s W[s,t] * xT[s,r], accumulated over s-tiles in PSUM,
    skipping all-zero below-diagonal blocks (272 of 512 matmuls).
  - bias is fused into the PSUM->SBUF copy on the scalar engine
    (activation Identity with per-partition bias).
  - host transposes each core's outT back and stacks.
"""

import numpy as np

import concourse.bass as bass
import concourse.mybir as mybir
import concourse.tile as tile
from concourse import bacc
from concourse.bass_utils import run_bass_kernel_spmd

F32 = mybir.dt.float32
F32R = mybir.dt.float32r

B = 8
E = 1024
S = 2048
DC = 1.0
N_CORES = 8
R = (B * E) // N_CORES      # rows per core = 1024
ST = S // 128               # 16 s-tiles of 128
TB = S // 512               # 4 t-blocks of 512
RB = R // 512               # 2 r-blocks of 512

# chunk list: (si, tb) with si <= 4*tb+3  (40 chunks)
CHUNKS = [(si, tb) for tb in range(TB) for si in range(min(ST, 4 * tb + 4))]
CHUNK_IDX = {c: i for i, c in enumerate(CHUNKS)}
N_CHUNKS = len(CHUNKS)

_PROGRAM = None  # (nc, ...) cache


def _build_program(repeats=1, no_wgen=False, no_store=False, no_xload=False,
                   po_bufs=4, wc_bufs=30, osb_bufs=4, xsplit=1):
    nc = bacc.Bacc("TRN2", target_bir_lowering=False, debug=False,
                   num_devices=N_CORES)

    xT_d = nc.declare_dram_parameter("xT", [S, R], F32, isOutput=False)
    wstat_d = nc.declare_dram_parameter("wstat", [N_CHUNKS, 2, 128], F32,
                                        isOutput=False)
    wmov_d = nc.declare_dram_parameter("wmov", [N_CHUNKS, 2, 512], F32,
                                       isOutput=False)
    masks_d = nc.declare_dram_parameter("masks", [4, 128, 512], F32,
                                        isOutput=False)
    biasT_d = nc.declare_dram_parameter("biasT", [128, ST], F32,
                                        isOutput=False)
    outT_d = nc.declare_dram_parameter("outT", [S, R], F32, isOutput=True)

    with tile.TileContext(nc) as tc:
        with (
            tc.tile_pool(name="xp", bufs=1) as xp,
            tc.tile_pool(name="cst", bufs=1) as cst,
            tc.tile_pool(name="wg", bufs=6) as wg,
            tc.tile_pool(name="wc", bufs=wc_bufs) as wcp,
            tc.tile_pool(name="osb", bufs=osb_bufs) as osb,
            tc.tile_pool(name="pw", bufs=2, space="PSUM") as pwp,
            tc.tile_pool(name="po", bufs=po_bufs, space="PSUM") as pop,
        ):
            mask_sb = []
            for m in range(4):
                mt = cst.tile([128, 512], F32, tag=f"mask{m}")
                nc.gpsimd.dma_start(mt[:], masks_d[m])
                mask_sb.append(mt)
            bias_sb = cst.tile([128, ST], F32, tag="bias")
            nc.gpsimd.dma_start(bias_sb[:], biasT_d[:])

            for rep in range(repeats):
              # resident x tiles: [128 s, 1024 r] per s-tile
              xs = []
              for si in range(ST):
                t = xp.tile([128, R], F32R, tag=f"x{si}", name=f"x{si}_{rep}")
                if not no_xload:
                    for xs_i in range(xsplit):
                        w0c = (R // xsplit) * xs_i
                        w1c = (R // xsplit) * (xs_i + 1)
                        nc.sync.dma_start(
                            t[:, w0c:w1c],
                            xT_d[128 * si:128 * (si + 1), w0c:w1c]
                            .bitcast(F32R))
                xs.append(t)
              def emit_wgen(tb):
                # generate W chunks (si, tb) for t-block tb
                w_sb = []
                for si in range(min(ST, 4 * tb + 4)):
                    w = wcp.tile([128, 512], F32R, tag="wc", name=f"w{tb}_{si}")
                    if no_wgen:
                        nc.gpsimd.memset(w[:], 0.0)
                    else:
                        ci = CHUNK_IDX[(si, tb)]
                        st = wg.tile([2, 128], F32R, tag="wstat", name="st")
                        nc.gpsimd.dma_start(st[:], wstat_d[ci].bitcast(F32R))
                        mv = wg.tile([2, 512], F32R, tag="wmov", name="mv")
                        nc.gpsimd.dma_start(mv[:], wmov_d[ci].bitcast(F32R))
                        psw = pwp.tile([128, 512], F32, tag="pw", name="psw")
                        nc.tensor.matmul(psw[:], st[:], mv[:], start=True,
                                         stop=True)
                        d2 = 4 * tb - si
                        if d2 <= 0:
                            nc.vector.tensor_mul(w[:], psw[:],
                                                 mask_sb[d2 + 3][:])
                        else:
                            nc.vector.tensor_copy(w[:], psw[:])
                    w_sb.append(w)
                return w_sb

              w_by_tb = {0: emit_wgen(0), 1: emit_wgen(1)}
              for tb in range(TB):
                w_sb = w_by_tb.pop(tb)
                for tjl in range(4):
                    tj = 4 * tb + tjl
                    out_sb = osb.tile([128, R], F32, tag="osb")
                    ps = [pop.tile([128, 512], F32, tag="po", name=f"po{rb}")
                          for rb in range(RB)]
                    for si in range(tj + 1):
                        lhsT = w_sb[si][:, 128 * tjl:128 * (tjl + 1)]
                        for rb in range(RB):
                            nc.tensor.matmul(
                                ps[rb][:], lhsT,
                                xs[si][:, 512 * rb:512 * (rb + 1)],
                                start=(si == 0), stop=(si == tj),
                            )
                    for rb in range(RB):
                        nc.scalar.activation(
                            out_sb[:, 512 * rb:512 * (rb + 1)], ps[rb][:],
                            mybir.ActivationFunctionType.Identity,
                            bias=bias_sb[:, tj:tj + 1],
                        )
                    if not no_store:
                        nc.sync.dma_start(
                            outT_d[128 * tj:128 * (tj + 1), :], out_sb[:])
                if tb + 2 < TB:
                    w_by_tb[tb + 2] = emit_wgen(tb + 2)

    nc.compile()
    return nc


def _host_prep(weight, bias, decay_value):
    w0 = weight[0].astype(np.float64)
    w1 = weight[1].astype(np.float64)
    d0 = float(np.clip(np.float32(decay_value[0, 0]), 0.9, 1.0))
    d1 = float(np.clip(np.float32(decay_value[1, 0]), 0.9, 1.0))
    ii = np.arange(128, dtype=np.float64)
    jj = np.arange(512, dtype=np.float64)

    wstat = np.zeros((N_CHUNKS, 2, 128), dtype=np.float32)
    wmov = np.zeros((N_CHUNKS, 2, 512), dtype=np.float32)
    for ci, (si, tb) in enumerate(CHUNKS):
        d2 = 4 * tb - si
        # W[i,j] = w0[i]*d0^(j-i) + w1[j]*d1^(j-i), j-i = 128*d2 + jj - ii
        wstat[ci, 0] = (w0[128 * si:128 * (si + 1)] * d0 ** (-ii / DC)
                        ).astype(np.float32)
        wstat[ci, 1] = (d1 ** ((128 * d2 - ii) / DC)).astype(np.float32)
        wmov[ci, 0] = (d0 ** ((128 * d2 + jj) / DC)).astype(np.float32)
        wmov[ci, 1] = (w1[512 * tb:512 * (tb + 1)] * d1 ** (jj / DC)
                       ).astype(np.float32)

    masks = np.zeros((4, 128, 512), dtype=np.float32)
    for m in range(4):
        d2 = m - 3
        masks[m] = (128 * d2 + jj[None, :] - ii[:, None] >= 0
                    ).astype(np.float32)

    biasT = np.ascontiguousarray(
        bias.astype(np.float32).reshape(ST, 128).T)
    return wstat, wmov, masks, biasT


def kernel(x, weight, bias, decay_value, index=0, recurrent=0, **_):
    global _PROGRAM
    x = np.asarray(x, dtype=np.float32)
    weight = np.asarray(weight, dtype=np.float32)
    bias = np.asarray(bias, dtype=np.float32)
    decay_value = np.asarray(decay_value, dtype=np.float32)

    if _PROGRAM is None:
        _PROGRAM = _build_program()
    nc = _PROGRAM

    wstat, wmov, masks, biasT = _host_prep(weight, bias, decay_value)

    x2 = x.reshape(B * E, S)
    in_maps = []
    for c in range(N_CORES):
        xT_c = np.ascontiguousarray(x2[R * c:R * (c + 1), :].T)
        in_maps.append({
            "xT": xT_c, "wstat": wstat, "wmov": wmov,
            "masks": masks, "biasT": biasT,
        })

    res = run_bass_kernel_spmd(nc, in_maps, core_ids=list(range(N_CORES)))
    out = np.empty((B * E, S), dtype=np.float32)
    for c in range(N_CORES):
        out[R * c:R * (c + 1), :] = res.results[c]["outT"].T
    return out.reshape(B, E, S)



# revision 9
# speedup vs baseline: 1.6452x; 1.6452x over previous
"""Trainium2 Bass kernel for CombinedRepeatCausalLinear (parallel forward).

Computes out[b,e,t] = sum_s x[b,e,s] * W[s,t] + bias[t] where
  W[s,t] = mask(t>=s) * (w0[s]*d0^(t-s) + w1[t]*d1^(t-s))
for S = 2048, x of shape (8, 1024, 2048) fp32.

W is generated by two first-order linear recurrences along t, so instead
of the dense causal GEMM (136 tile-pairs) we run a blocked scan:

  out[r, 128k+tau] = sum_{sig<=tau} x[r,128k+sig] * Wloc_k[sig,tau]   (local)
                   + d0^(tau+1) * A[r,128k-1]                          (carry A)
                   + w1[t] * d1^(tau+1) * C[r,128k-1]                  (carry C)
  A, C = the two running scan states, reconstructed from per-chunk
  summaries E via a tiny 32x32 "transfer" matmul.

Per core (data-parallel over B*E rows, 1024 rows/core, xT layout so the
contraction dim is on SBUF partitions):
  - 16 local matmuls  [K=128, M=128, N=512] x 2 r-blocks   (diag chunks)
  - 16 summary matmuls [K=128, M=32, N=512] x 2  -> E  (accumulated PSUM)
  -  1 transfer matmul [K=32, M=32, N=512]  x 2  -> carries
  - 16 carry-inject matmuls [K=2, M=128, N=512] x 2 into the local PSUM
  = 98 PE streams of 512 rows vs 312 for the dense version.

All matmul operands are bf16 (fp32 PSUM accumulation); x is cast to bf16
on the host, halving HBM traffic. Output is stored bf16 and upcast on
host. bias is fused into the PSUM->SBUF copy on the scalar engine.
"""

import numpy as np
import ml_dtypes

import concourse.bass as bass
import concourse.mybir as mybir
import concourse.tile as tile
from concourse import bacc
from concourse.bass_utils import run_bass_kernel_spmd

F32 = mybir.dt.float32
BF16 = mybir.dt.bfloat16
BF = ml_dtypes.bfloat16

B = 8
E = 1024
S = 2048
DC = 1.0
N_CORES = 8
R = (B * E) // N_CORES      # rows per core = 1024
CH = 128                    # chunk (scan block) size
NK = S // CH                # 16 chunks
RB = R // 512               # 2 r-blocks of 512
NEARLY = 3                  # chunks whose local matmul is emitted early

_PROGRAM = None


def _build_program():
    nc = bacc.Bacc("TRN2", target_bir_lowering=False, debug=False,
                   num_devices=N_CORES)

    xT_d = nc.declare_dram_parameter("xT", [S, R], BF16, isOutput=False)
    loc_d = nc.declare_dram_parameter("loc", [NK, CH, CH], BF16,
                                      isOutput=False)
    ssum_d = nc.declare_dram_parameter("ssum", [NK, CH, 2 * NK], BF16,
                                       isOutput=False)
    tmat_d = nc.declare_dram_parameter("tmat", [2 * NK, 2 * NK], BF16,
                                       isOutput=False)
    inj_d = nc.declare_dram_parameter("inj", [NK, 2 * NK, CH], BF16,
                                      isOutput=False)
    biasT_d = nc.declare_dram_parameter("biasT", [CH, NK], F32,
                                        isOutput=False)
    outT_d = nc.declare_dram_parameter("outT", [S, R], BF16, isOutput=True)

    with tile.TileContext(nc) as tc:
        with (
            tc.tile_pool(name="xp", bufs=1) as xp,
            tc.tile_pool(name="cst", bufs=1) as cst,
            tc.tile_pool(name="sb", bufs=1) as sbp,
            tc.tile_pool(name="osb", bufs=4) as osb,
            tc.tile_pool(name="pe", bufs=2, space="PSUM") as pep,
            tc.tile_pool(name="po", bufs=6, space="PSUM") as pop,
        ):
            # ---- constants ----
            loc_sb, ssum_sb, inj_sb = [], [], []
            for k in range(NK):
                t = cst.tile([CH, CH], BF16, tag=f"loc{k}")
                nc.gpsimd.dma_start(t[:], loc_d[k])
                loc_sb.append(t)
                t = cst.tile([CH, 2 * NK], BF16, tag=f"ssum{k}")
                nc.gpsimd.dma_start(t[:], ssum_d[k])
                ssum_sb.append(t)
                t = cst.tile([2 * NK, CH], BF16, tag=f"inj{k}")
                nc.gpsimd.dma_start(t[:], inj_d[k])
                inj_sb.append(t)
            tmat_sb = cst.tile([2 * NK, 2 * NK], BF16, tag="tmat")
            nc.gpsimd.dma_start(tmat_sb[:], tmat_d[:])
            bias_sb = cst.tile([CH, NK], F32, tag="bias")
            nc.gpsimd.dma_start(bias_sb[:], biasT_d[:])

            # ---- x loads: two rings (sync: rb0 half, gpsimd: rb1 half) ----
            xs = []
            for k in range(NK):
                t = xp.tile([CH, R], BF16, tag=f"x{k}")
                nc.sync.dma_start(t[:, 0:512], xT_d[CH * k:CH * (k + 1), 0:512])
                nc.gpsimd.dma_start(t[:, 512:1024],
                                    xT_d[CH * k:CH * (k + 1), 512:1024])
                xs.append(t)

            # ---- phase A: summaries (+ a few early locals) ----
            E_ps = [pep.tile([2 * NK, 512], F32, tag="E", name=f"E{rb}")
                    for rb in range(RB)]
            po_tiles = {}
            for k in range(NK):
                for rb in range(RB):
                    nc.tensor.matmul(E_ps[rb][:], ssum_sb[k][:],
                                     xs[k][:, 512 * rb:512 * (rb + 1)],
                                     start=(k == 0), stop=(k == NK - 1))
                if k < NEARLY:
                    for rb in range(RB):
                        ps = pop.tile([CH, 512], F32, tag="po",
                                      name=f"po{k}_{rb}")
                        nc.tensor.matmul(ps[:], loc_sb[k][:],
                                         xs[k][:, 512 * rb:512 * (rb + 1)],
                                         start=True, stop=False)
                        po_tiles[(k, rb)] = ps

            # ---- phase B: E -> carries ----
            E_sb, carry_sb = [], []
            for rb in range(RB):
                t = sbp.tile([2 * NK, 512], BF16, tag=f"esb{rb}")
                if rb == 0:
                    nc.vector.tensor_copy(t[:], E_ps[rb][:])
                else:
                    nc.scalar.activation(t[:], E_ps[rb][:],
                                         mybir.ActivationFunctionType.Copy)
                E_sb.append(t)
            carry_ps = [pep.tile([2 * NK, 512], F32, tag="E",
                                 name=f"carry{rb}") for rb in range(RB)]
            for rb in range(RB):
                nc.tensor.matmul(carry_ps[rb][:], tmat_sb[:], E_sb[rb][:],
                                 start=True, stop=True)
            for rb in range(RB):
                t = sbp.tile([2 * NK, 512], BF16, tag=f"csb{rb}")
                if rb == 0:
                    nc.vector.tensor_copy(t[:], carry_ps[rb][:])
                else:
                    nc.scalar.activation(t[:], carry_ps[rb][:],
                                         mybir.ActivationFunctionType.Copy)
                carry_sb.append(t)

            # ---- phase C: locals + carry injection + bias copy + store ----
            for k in range(NK):
                out_sb = osb.tile([CH, R], BF16, tag="osb")
                for rb in range(RB):
                    ps = po_tiles.pop((k, rb), None)
                    if ps is None:
                        ps = pop.tile([CH, 512], F32, tag="po",
                                      name=f"po{k}_{rb}")
                        nc.tensor.matmul(ps[:], loc_sb[k][:],
                                         xs[k][:, 512 * rb:512 * (rb + 1)],
                                         start=True, stop=False)
                    nc.tensor.matmul(ps[:], inj_sb[k][:], carry_sb[rb][:],
                                     start=False, stop=True)
                    nc.scalar.activation(
                        out_sb[:, 512 * rb:512 * (rb + 1)], ps[:],
                        mybir.ActivationFunctionType.Identity,
                        bias=bias_sb[:, k:k + 1],
                    )
                nc.sync.dma_start(outT_d[CH * k:CH * (k + 1), :], out_sb[:])

    nc.compile()
    return nc


def _host_prep(weight, bias, decay_value):
    w0 = weight[0].astype(np.float64)
    w1 = weight[1].astype(np.float64)
    d0 = float(np.clip(np.float32(decay_value[0, 0]), 0.9, 1.0))
    d1 = float(np.clip(np.float32(decay_value[1, 0]), 0.9, 1.0))
    sig = np.arange(CH, dtype=np.float64)
    tau = np.arange(CH, dtype=np.float64)

    loc = np.zeros((NK, CH, CH), dtype=np.float64)
    ssum = np.zeros((NK, CH, 2 * NK), dtype=np.float64)
    inj = np.zeros((NK, 2 * NK, CH), dtype=np.float64)
    tmat = np.zeros((2 * NK, 2 * NK), dtype=np.float64)
    with np.errstate(under="ignore"):
        m2 = tau[None, :] >= sig[:, None]
        p2 = np.where(m2, tau[None, :] - sig[:, None], 0.0) / DC
        for k in range(NK):
            w0c = w0[CH * k:CH * (k + 1)]
            w1c = w1[CH * k:CH * (k + 1)]
            loc[k] = np.where(m2, w0c[:, None] * d0 ** p2
                              + w1c[None, :] * d1 ** p2, 0.0)
            ssum[k][:, 2 * k] = w0c * d0 ** ((CH - 1 - sig) / DC)
            ssum[k][:, 2 * k + 1] = d1 ** ((CH - 1 - sig) / DC)
            inj[k][2 * k] = d0 ** ((tau + 1) / DC)
            inj[k][2 * k + 1] = w1c * d1 ** ((tau + 1) / DC)
            for kp in range(k):
                tmat[2 * kp, 2 * k] = d0 ** (CH * (k - kp - 1) / DC)
                tmat[2 * kp + 1, 2 * k + 1] = d1 ** (CH * (k - kp - 1) / DC)

    biasT = np.ascontiguousarray(
        bias.astype(np.float32).reshape(NK, CH).T)
    return (loc.astype(BF), ssum.astype(BF), tmat.astype(BF),
            inj.astype(BF), biasT)


def _make_in_maps(x, weight, bias, decay_value):
    loc, ssum, tmat, inj, biasT = _host_prep(weight, bias, decay_value)
    x2 = np.asarray(x, dtype=np.float32).reshape(B * E, S)
    in_maps = []
    for c in range(N_CORES):
        xT_c = np.ascontiguousarray(x2[R * c:R * (c + 1), :].T.astype(BF))
        in_maps.append({
            "xT": xT_c, "loc": loc, "ssum": ssum, "tmat": tmat,
            "inj": inj, "biasT": biasT,
        })
    return in_maps


def kernel(x, weight, bias, decay_value, index=0, recurrent=0, **_):
    global _PROGRAM
    x = np.asarray(x, dtype=np.float32)
    weight = np.asarray(weight, dtype=np.float32)
    bias = np.asarray(bias, dtype=np.float32)
    decay_value = np.asarray(decay_value, dtype=np.float32)

    if _PROGRAM is None:
        _PROGRAM = _build_program()
    nc = _PROGRAM

    in_maps = _make_in_maps(x, weight, bias, decay_value)

    res = run_bass_kernel_spmd(nc, in_maps, core_ids=list(range(N_CORES)))
    out = np.empty((B * E, S), dtype=np.float32)
    for c in range(N_CORES):
        out[R * c:R * (c + 1), :] = res.results[c]["outT"].astype(np.float32).T
    return out.reshape(B, E, S)


# revision 12
# speedup vs baseline: 2.4158x; 1.4684x over previous
"""Trainium2 Bass kernel for CombinedRepeatCausalLinear (parallel forward).

Computes out[b,e,t] = sum_s x[b,e,s] * W[s,t] + bias[t] where
  W[s,t] = mask(t>=s) * (w0[s]*d0^(t-s) + w1[t]*d1^(t-s))
for S = 2048, x of shape (8, 1024, 2048) fp32.

W is generated by two first-order linear recurrences along t, so instead
of the dense causal GEMM we run a blocked scan (per core, data-parallel
over B*E rows, 1024 rows/core, xT layout so the contraction dim is on
SBUF partitions):

  out[r, 128k+tau] = sum_{sig<=tau} x[r,128k+sig] * Wloc_k[sig,tau]
                   + d0^(tau+1) * A[r,128k-1] + w1[t]*d1^(tau+1)*C[r,128k-1]

with the scan states A, C rebuilt from per-chunk summaries E via a tiny
32x32 transfer matmul. Per r-block of 512 rows: 16 local matmuls
[K=128,M=128,N=512] + 16 summary matmuls [K=128,M=32,N=512] (PSUM
accumulated) + 1 transfer matmul + 16 zero-padded carry-inject matmuls
[K=32,M=128,N=512] = 49 PE streams; 98 total vs 312 for the dense
version.

Schedule: the two r-halves are pipelined - rb0's column-halves of x load
first, so rb0's summaries/carries/output chunks run while rb1 is still
loading, keeping the PE stream dense (full p-state clock). All matmul
operands are bf16 (fp32 PSUM accumulation); x is cast to bf16 on the
host, halving HBM traffic; output is stored bf16 and upcast on host.
Constants ship as 2 packed DMAs; x loads/output stores ride only the
fast SP/Activation DMA rings (gpsimd triggers are ~800ns each). The
PSUM->SBUF bias-apply copies alternate scalar/vector engines.
"""

import numpy as np
import ml_dtypes

import concourse.bass as bass
import concourse.mybir as mybir
import concourse.tile as tile
from concourse import bacc
from concourse.bass_utils import run_bass_kernel_spmd

F32 = mybir.dt.float32
BF16 = mybir.dt.bfloat16
BF = ml_dtypes.bfloat16

B = 8
E = 1024
S = 2048
DC = 1.0
N_CORES = 8
R = (B * E) // N_CORES      # rows per core = 1024
CH = 128                    # chunk (scan block) size
NK = S // CH                # 16 chunks
RB = 2                      # r-blocks of 512
NEARLY = 3                  # chunks whose local matmul is emitted early

# packed-constant free-dim offsets (bf16, partitions 0:128 / 0:32)
CA_LOC = 0                  # cstA[:, 128k:128(k+1)]        loc_k  [128,128]
CA_SSUM = NK * CH           # cstA[:, 2048+32k:...]         ssum_k [128,32]
CA_W = NK * CH + NK * 2 * NK
CB_INJ = 0                  # cstB[0:32, 128k:128(k+1)]     inj_k  [32,128]
CB_TMAT = NK * CH           # cstB[0:32, 2048:2080]         tmat   [32,32]
CB_W = NK * CH + 2 * NK

_PROGRAM = None


def _build_program():
    nc = bacc.Bacc("TRN2", target_bir_lowering=False, debug=False,
                   num_devices=N_CORES)

    xT_d = nc.declare_dram_parameter("xT", [S, R], BF16, isOutput=False)
    cA_d = nc.declare_dram_parameter("cA", [CH, CA_W], BF16, isOutput=False)
    cB_d = nc.declare_dram_parameter("cB", [2 * NK, CB_W], BF16,
                                     isOutput=False)
    biasT_d = nc.declare_dram_parameter("biasT", [CH, NK], F32,
                                        isOutput=False)
    outT_d = nc.declare_dram_parameter("outT", [S, R], BF16, isOutput=True)

    ACT = mybir.ActivationFunctionType

    with tile.TileContext(nc) as tc:
        with (
            tc.tile_pool(name="xp", bufs=1) as xp,
            tc.tile_pool(name="cst", bufs=1) as cst,
            tc.tile_pool(name="sb", bufs=1) as sbp,
            tc.tile_pool(name="osb", bufs=16) as osb,
            tc.tile_pool(name="pe", bufs=2, space="PSUM") as pep,
            tc.tile_pool(name="po", bufs=6, space="PSUM") as pop,
        ):
            # ---- constants: two packed DMAs + bias, on gpsimd ----
            cstA = cst.tile([CH, CA_W], BF16, tag="cA")
            nc.gpsimd.dma_start(cstA[:], cA_d[:])
            cstB = cst.tile([2 * NK, CB_W], BF16, tag="cB")
            nc.gpsimd.dma_start(cstB[:], cB_d[:])
            bias_sb = cst.tile([CH, NK], F32, tag="bias")
            nc.gpsimd.dma_start(bias_sb[:], biasT_d[:])

            def loc_w(k):
                return cstA[:, CA_LOC + CH * k:CA_LOC + CH * (k + 1)]

            def ssum_w(k):
                return cstA[:, CA_SSUM + 2 * NK * k:CA_SSUM + 2 * NK * (k + 1)]

            def inj_w(k):
                return cstB[:, CB_INJ + CH * k:CB_INJ + CH * (k + 1)]

            tmat_w = cstB[:, CB_TMAT:CB_TMAT + 2 * NK]

            # ---- x loads: rb-major, alternating SP/Activation rings ----
            xs = [xp.tile([CH, R], BF16, tag=f"x{k}", name=f"x{k}")
                  for k in range(NK)]
            for rb in range(RB):
                for k in range(NK):
                    eng = nc.sync if k % 2 == 0 else nc.scalar
                    eng.dma_start(
                        xs[k][:, 512 * rb:512 * (rb + 1)],
                        xT_d[CH * k:CH * (k + 1), 512 * rb:512 * (rb + 1)])

            E_ps = {}
            carry_sb = {}
            po_tiles = {}
            out_sb = {}

            def emit_summary(k, rb):
                nc.tensor.matmul(E_ps[rb][:], ssum_w(k),
                                 xs[k][:, 512 * rb:512 * (rb + 1)],
                                 start=(k == 0), stop=(k == NK - 1))

            def emit_local(k, rb):
                ps = pop.tile([CH, 512], F32, tag="po", name=f"po{k}_{rb}")
                nc.tensor.matmul(ps[:], loc_w(k),
                                 xs[k][:, 512 * rb:512 * (rb + 1)],
                                 start=True, stop=False)
                po_tiles[(k, rb)] = ps

            def emit_carries(rb):
                # E -> SBUF -> transfer matmul -> carries -> SBUF (vector)
                e_sb = sbp.tile([2 * NK, 512], BF16, tag=f"esb{rb}")
                nc.vector.tensor_copy(e_sb[:], E_ps[rb][:])
                c_ps = pep.tile([2 * NK, 512], F32, tag="pe",
                                name=f"carry{rb}")
                nc.tensor.matmul(c_ps[:], tmat_w, e_sb[:],
                                 start=True, stop=True)
                c_sb = sbp.tile([2 * NK, 512], BF16, tag=f"csb{rb}")
                nc.vector.tensor_copy(c_sb[:], c_ps[:])
                carry_sb[rb] = c_sb

            def emit_out(k, rb):
                # carry inject into the local PSUM, then bias-copy to SBUF
                ps = po_tiles.pop((k, rb))
                nc.tensor.matmul(ps[:], inj_w(k), carry_sb[rb][:],
                                 start=False, stop=True)
                if k not in out_sb:
                    out_sb[k] = osb.tile([CH, R], BF16, tag="osb",
                                         name=f"o{k}")
                dst = out_sb[k][:, 512 * rb:512 * (rb + 1)]
                if k % 2 == 0:
                    nc.scalar.activation(dst, ps[:], ACT.Identity,
                                         bias=bias_sb[:, k:k + 1])
                else:
                    nc.vector.tensor_scalar_add(dst, ps[:],
                                                bias_sb[:, k:k + 1])

            def emit_store(k):
                eng = nc.sync if k % 2 == 0 else nc.scalar
                eng.dma_start(outT_d[CH * k:CH * (k + 1), :], out_sb[k][:])

            # ---- phase A(rb0): summaries + early locals ----
            E_ps[0] = pep.tile([2 * NK, 512], F32, tag="pe", name="E0")
            for k in range(NK):
                emit_summary(k, 0)
                if k < NEARLY:
                    emit_local(k, 0)
            # ---- B(rb0) ----
            emit_carries(0)
            # ---- C(rb0) interleaved with A(rb1) ----
            E_ps[1] = pep.tile([2 * NK, 512], F32, tag="pe", name="E1")
            for k in range(NK):
                if (k, 0) not in po_tiles:
                    emit_local(k, 0)
                emit_out(k, 0)
                emit_summary(k, 1)
                if k < NEARLY:
                    emit_local(k, 1)
            # ---- B(rb1) ----
            emit_carries(1)
            # ---- C(rb1) + stores ----
            for k in range(NK):
                if (k, 1) not in po_tiles:
                    emit_local(k, 1)
                emit_out(k, 1)
                emit_store(k)

    nc.compile()
    return nc


def _host_prep(weight, bias, decay_value):
    w0 = weight[0].astype(np.float64)
    w1 = weight[1].astype(np.float64)
    d0 = float(np.clip(np.float32(decay_value[0, 0]), 0.9, 1.0))
    d1 = float(np.clip(np.float32(decay_value[1, 0]), 0.9, 1.0))
    sig = np.arange(CH, dtype=np.float64)
    tau = np.arange(CH, dtype=np.float64)

    cA = np.zeros((CH, CA_W), dtype=np.float64)
    cB = np.zeros((2 * NK, CB_W), dtype=np.float64)
    with np.errstate(under="ignore"):
        m2 = tau[None, :] >= sig[:, None]
        p2 = np.where(m2, tau[None, :] - sig[:, None], 0.0) / DC
        for k in range(NK):
            w0c = w0[CH * k:CH * (k + 1)]
            w1c = w1[CH * k:CH * (k + 1)]
            cA[:, CA_LOC + CH * k:CA_LOC + CH * (k + 1)] = np.where(
                m2, w0c[:, None] * d0 ** p2 + w1c[None, :] * d1 ** p2, 0.0)
            sk = CA_SSUM + 2 * NK * k
            cA[:, sk + 2 * k] = w0c * d0 ** ((CH - 1 - sig) / DC)
            cA[:, sk + 2 * k + 1] = d1 ** ((CH - 1 - sig) / DC)
            cB[2 * k, CB_INJ + CH * k:CB_INJ + CH * (k + 1)] = \
                d0 ** ((tau + 1) / DC)
            cB[2 * k + 1, CB_INJ + CH * k:CB_INJ + CH * (k + 1)] = \
                w1c * d1 ** ((tau + 1) / DC)
            for kp in range(k):
                cB[2 * kp, CB_TMAT + 2 * k] = d0 ** (CH * (k - kp - 1) / DC)
                cB[2 * kp + 1, CB_TMAT + 2 * k + 1] = \
                    d1 ** (CH * (k - kp - 1) / DC)

    biasT = np.ascontiguousarray(bias.astype(np.float32).reshape(NK, CH).T)
    return cA.astype(BF), cB.astype(BF), biasT


def _make_in_maps(x, weight, bias, decay_value):
    cA, cB, biasT = _host_prep(weight, bias, decay_value)
    x2 = np.asarray(x, dtype=np.float32).reshape(B * E, S)
    in_maps = []
    for c in range(N_CORES):
        xT_c = np.ascontiguousarray(x2[R * c:R * (c + 1), :].T.astype(BF))
        in_maps.append({"xT": xT_c, "cA": cA, "cB": cB, "biasT": biasT})
    return in_maps


def kernel(x, weight, bias, decay_value, index=0, recurrent=0, **_):
    global _PROGRAM
    x = np.asarray(x, dtype=np.float32)
    weight = np.asarray(weight, dtype=np.float32)
    bias = np.asarray(bias, dtype=np.float32)
    decay_value = np.asarray(decay_value, dtype=np.float32)

    if _PROGRAM is None:
        _PROGRAM = _build_program()
    nc = _PROGRAM

    in_maps = _make_in_maps(x, weight, bias, decay_value)

    res = run_bass_kernel_spmd(nc, in_maps, core_ids=list(range(N_CORES)))
    out = np.empty((B * E, S), dtype=np.float32)
    for c in range(N_CORES):
        out[R * c:R * (c + 1), :] = res.results[c]["outT"].astype(np.float32).T
    return out.reshape(B, E, S)
